# revision 30
# baseline (speedup 1.0000x reference)
"""Trainium2 Bass kernel for nn_MatchNet (MLP forward + 60-iter batched PDHG LP solve).

Data-parallel over 8 NeuronCores: batch 2048 -> 256 rows/core (2 b-tiles of 128).
MLP runs in float32r (1 cyc/row matmuls at N=256). PDHG states are fp16 in
N layout [batch, n] with alpha-scaled prox states so every constant folds into
matmul weights or activation scale/bias.

Math per core (n=512 structures, m=64 combos, tau=sigma=0.9/L, alpha=tau*sigma):
    Z = relu(relu(relu(X@W1+b1)@W2+b2)@W3+b3)
    states: q = tau*y2 (fp16), aeb = alpha*(xbar - Z), E = alpha*(x - Z + tau),
            pc = p + cSZB with p = tau*y1, cSZB = alpha*(S@Z^T - B^T)
    iter:
      w   = q + naZ            (naZ = -alpha*Z)
      h   = w - aeb
      qn  = max(h, 0)
      aebT = transpose(aeb)                       (PE + evac)
      ps1 = I64@pc + sum_c S^T_c @ aebT_c        (PSUM [64,128])
      p   = relu(ps1);  pcn = p + cSZB
      NS3 = p@(a16*S) + (-a16*I)@qn + (-I)@E     (PSUM [128,512] = -alpha*d)
      n2  = sum(NS3^2); rr = 1/max(n2,eps); nr = sqrt(tau^2 alpha^2 rr)
      ns  = min(nr-1, 0)                          (= -s)
      En  = ns*NS3 + alpha*tau                    (Act scale-ptr; = E_new)
      tmp = 2*En - alpha*tau;  aebn = tmp - E
    out x = Z + E/alpha - tau
"""

import numpy as np

N_STRUCTS = 512
N_COMBOS = 64
HID = 1024
N_ITERS = 60
N_CORES = 8
B_FULL = 2048
BC = B_FULL // N_CORES  # 256 batch rows per core
NB = BC // 128  # 2 batch sub-tiles
NF = N_STRUCTS // 128  # 4 feature chunks

# engine assignment knobs
CFG = {
    "w": "pool",      # w = q + naZ
    "h": "dve",       # h = w - aeb
    "qn": "dve",      # qn = max(h,0)
    "tev": "split",   # aebT evacuation: act | dve | split
    "p": "act",       # p = relu(ps1)
    "pc": "dve",      # pcn = p + cSZB
    "n2": "act",  # act Square+accum (DVE amr cannot read PSUM twice)
    "en": "act",      # En from PSUM: act | dve
}

_BUILD_CACHE = {}


def _power_L(S: np.ndarray) -> float:
    """Replicates reference.py's power iteration for ||K||_2 in float32."""
    S = S.astype(np.float32)
    n = S.shape[1]
    v = np.full((n,), 1.0 / np.sqrt(n), np.float32)
    for _ in range(30):
        v2 = (S.T @ (S @ v) + v).astype(np.float32)
        v = (v2 / np.float32(np.linalg.norm(v2))).astype(np.float32)
    L = np.sqrt(np.vdot(v, (S.T @ (S @ v) + v).astype(np.float32)))
    return float(L)


def _build_nc(tau: float, sigma: float):
    import contextlib

    import concourse.bacc as bacc
    import concourse.mybir as mybir
    import concourse.tile as tile

    f32 = mybir.dt.float32
    f32r = mybir.dt.float32r
    f16 = mybir.dt.float16
    AF = mybir.ActivationFunctionType
    ALU = mybir.AluOpType
    alpha = float(np.float32(tau) * np.float32(sigma))
    atau = float(np.float32(alpha) * np.float32(tau))
    t2a2 = float((np.float32(tau) * np.float32(alpha)) ** 2)
    dsq_scale = float(1.0 / t2a2)

    nc = bacc.Bacc("TRN2", target_bir_lowering=False, debug=False)

    def creg(v):
        key = (f32, v)
        if key not in nc.const_aps.aps:
            t = nc.alloc_sbuf_tensor(f"constx-{v}", [128, 1], f32)
            nc.gpsimd.memset(t.ap(), v)
            nc.const_aps.aps[key] = t.ap()
        return v

    creg(atau)
    creg(-atau)
    creg(-tau)
    creg(1e-6)

    # ---- DRAM I/O (per-core shapes) ----
    d_XT = nc.dram_tensor("xt", [N_COMBOS, BC], f32r, kind="ExternalInput")
    d_W1 = nc.dram_tensor("w1", [N_COMBOS, HID], f32r, kind="ExternalInput")
    d_b1 = nc.dram_tensor("b1r", [128, 8], f32, kind="ExternalInput")
    d_W2 = nc.dram_tensor("w2", [HID, HID], f16, kind="ExternalInput")
    d_b2 = nc.dram_tensor("b2r", [128, 8], f32, kind="ExternalInput")
    d_W3 = nc.dram_tensor("w3", [HID, N_STRUCTS], f16, kind="ExternalInput")
    d_b3 = nc.dram_tensor("b3r", [128, 4], f32, kind="ExternalInput")
    d_aST = nc.dram_tensor("ast", [128, NF * N_COMBOS], f32r, kind="ExternalInput")
    d_ST16 = nc.dram_tensor("st16", [128, NF * N_COMBOS], f16, kind="ExternalInput")
    d_AS16 = nc.dram_tensor("as16", [N_COMBOS, N_STRUCTS], f16, kind="ExternalInput")
    d_nAI16 = nc.dram_tensor("nai16", [128, 128], f16, kind="ExternalInput")
    d_nI16 = nc.dram_tensor("ni16", [128, 128], f16, kind="ExternalInput")
    d_I64 = nc.dram_tensor("i64_16", [N_COMBOS, N_COMBOS], f16, kind="ExternalInput")
    d_I16 = nc.dram_tensor("i16", [128, 128], f16, kind="ExternalInput")
    d_Ir = nc.dram_tensor("identr", [128, 128], f32r, kind="ExternalInput")
    d_out = nc.dram_tensor("out", [BC, N_STRUCTS], f32, kind="ExternalOutput")

    FW = N_STRUCTS  # 512 per-b tile width

    with tile.TileContext(nc) as tc:
        stack = contextlib.ExitStack()
        with stack:
            cpool = stack.enter_context(tc.tile_pool(name="consts", bufs=1))

            def cload(dram, shape, tag, dt):
                t = cpool.tile(shape, dt, tag=tag)
                nc.sync.dma_start(t[:], dram.ap())
                return t

            XT = cload(d_XT, [N_COMBOS, BC], "xt", f32r)
            W1 = cload(d_W1, [N_COMBOS, HID], "w1", f32r)
            b1r = cload(d_b1, [128, 8], "b1r", f32)

            # ---- MLP forward (float32r, T layout) ----
            zt = []  # Z^T tiles [128, BC] x4, f32r
            with (
                tc.tile_pool(name="mlp_sb", bufs=1) as mpool,
                tc.tile_pool(name="mlp_ps", bufs=1, space="PSUM") as mpsum,
            ):
                W2 = []
                for k in range(8):
                    t = mpool.tile([128, HID], f16, tag=f"w2_{k}", name=f"w2_{k}")
                    nc.sync.dma_start(t[:], d_W2.ap()[k * 128 : (k + 1) * 128, :])
                    W2.append(t)
                b2r = cload(d_b2, [128, 8], "b2r", f32)
                W3 = []
                for k in range(8):
                    t = mpool.tile([128, N_STRUCTS], f16, tag=f"w3_{k}", name=f"w3_{k}")
                    nc.sync.dma_start(t[:], d_W3.ap()[k * 128 : (k + 1) * 128, :])
                    W3.append(t)
                b3r = cload(d_b3, [128, 4], "b3r", f32)
                aST = cload(d_aST, [128, NF * N_COMBOS], "ast", f32r)
                Ir = cload(d_Ir, [128, 128], "identr", f32r)
                ST16 = cload(d_ST16, [128, NF * N_COMBOS], "st16", f16)
                AS16 = cload(d_AS16, [N_COMBOS, N_STRUCTS], "as16", f16)
                nAI16 = cload(d_nAI16, [128, 128], "nai16", f16)
                nI16 = cload(d_nI16, [128, 128], "ni16", f16)
                I64 = cload(d_I64, [N_COMBOS, N_COMBOS], "i64_16", f16)
                I16 = cload(d_I16, [128, 128], "i16", f16)
                z1t = []
                for t in range(8):
                    ps = mpsum.tile([128, BC], f32, tag=f"zmm{t}")
                    nc.tensor.matmul(
                        ps[:], W1[:, t * 128 : (t + 1) * 128], XT[:], start=True, stop=True
                    )
                    sb = mpool.tile([128, BC], f16, tag=f"z1_{t}")
                    nc.scalar.activation(sb[:], ps[:], AF.Relu, bias=b1r[:, t : t + 1])
                    z1t.append(sb)
                # k-major: PE consumes each W2/W3 chunk as its DMA lands
                zps2 = [
                    mpsum.tile([128, BC], f32, tag=f"zmm{t}", name=f"zmm{t}")
                    for t in range(8)
                ]
                for k in range(8):
                    for t in range(8):
                        nc.tensor.matmul(
                            zps2[t][:],
                            W2[k][:, t * 128 : (t + 1) * 128],
                            z1t[k][:],
                            start=(k == 0),
                            stop=(k == 7),
                        )
                z2t = []
                for t in range(8):
                    sb = mpool.tile([128, BC], f16, tag=f"z2_{t}")
                    nc.scalar.activation(sb[:], zps2[t][:], AF.Relu, bias=b2r[:, t : t + 1])
                    z2t.append(sb)
                zps3 = [
                    mpsum.tile([128, BC], f32, tag=f"zmm{c}", name=f"z3mm{c}")
                    for c in range(NF)
                ]
                for k in range(8):
                    for c in range(NF):
                        nc.tensor.matmul(
                            zps3[c][:],
                            W3[k][:, c * 128 : (c + 1) * 128],
                            z2t[k][:],
                            start=(k == 0),
                            stop=(k == 7),
                        )
                for c in range(NF):
                    sb = cpool.tile([128, BC], f32r, tag=f"zt_{c}")
                    nc.scalar.activation(sb[:], zps3[c][:], AF.Relu, bias=b3r[:, c : c + 1])
                    zt.append(sb)

            # ---- PDHG setup ----
            spool = stack.enter_context(tc.tile_pool(name="setup", bufs=1))
            with tc.tile_pool(name="pd_ps", bufs=1, space="PSUM") as ppool:
                # cSZB16 = alpha*(S@Z^T - B^T)   [64, BC] fp16
                ps = ppool.tile([N_COMBOS, BC], f32, tag="py1")
                for c in range(NF):
                    nc.tensor.matmul(
                        ps[:], aST[:, c * 64 : (c + 1) * 64], zt[c][:],
                        start=(c == 0), stop=False,
                    )
                naI64 = spool.tile([N_COMBOS, N_COMBOS], f32r, tag="nai64")
                nc.scalar.activation(naI64[:], Ir[:64, :64].bitcast(f32), AF.Copy, scale=-alpha)
                nc.tensor.matmul(ps[:], naI64[:], XT[:], start=False, stop=True)
                cSZB = spool.tile([N_COMBOS, BC], f16, tag="cszb")
                nc.scalar.activation(cSZB[:], ps[:], AF.Copy)

                # Z per-b in N layout (f32) via PE transposes
                Zf = []
                for b in range(NB):
                    psz = ppool.tile([128, FW], f32r, tag=f"pz{b}")
                    for c in range(NF):
                        nc.tensor.transpose(
                            psz[:, c * 128 : (c + 1) * 128],
                            zt[c][:, b * 128 : (b + 1) * 128],
                            Ir[:],
                        )
                    zb = spool.tile([128, FW], f32, tag=f"zn{b}")
                    nc.scalar.activation(zb[:], psz[:].bitcast(f32), AF.Copy)
                    Zf.append(zb)

            # constants + state init (fp16)
            creg(0.0)
            naZ = []
            for b in range(NB):
                t = spool.tile([128, FW], f16, tag=f"naz{b}")
                if b == 0:
                    nc.scalar.activation(t[:], Zf[b][:], AF.Copy, scale=-alpha, bias=0.0)
                else:
                    nc.vector.tensor_scalar(t[:], Zf[b][:], -alpha, 0.0, op0=ALU.mult, op1=ALU.add)
                naZ.append(t)

            # ---- PDHG state pools ----
            e_pool = stack.enter_context(tc.tile_pool(name="ep", bufs=3))
            aeb_pool = stack.enter_context(tc.tile_pool(name="aebp", bufs=3))
            q_pool = stack.enter_context(tc.tile_pool(name="qp", bufs=3))
            p_pool = stack.enter_context(tc.tile_pool(name="pp", bufs=3))
            sc_pool = stack.enter_context(tc.tile_pool(name="scratch", bufs=3))
            ps_T = stack.enter_context(tc.tile_pool(name="ps_T", bufs=1, space="PSUM"))
            ps_y1 = stack.enter_context(tc.tile_pool(name="ps_y1", bufs=2, space="PSUM"))
            ps_3 = stack.enter_context(tc.tile_pool(name="ps_3", bufs=2, space="PSUM"))

            E, aeb, q, pc = [], [], [], []
            for b in range(NB):
                t = e_pool.tile([128, FW], f16, tag=f"e{b}")
                if b == 0:
                    nc.scalar.activation(t[:], Zf[b][:], AF.Copy, scale=-alpha, bias=atau)
                else:
                    nc.vector.tensor_scalar(t[:], Zf[b][:], -alpha, atau, op0=ALU.mult, op1=ALU.add)
                E.append(t)
                aeb.append(None)  # iteration 0 reads alpha*Z^T from zt directly
                t = q_pool.tile([128, FW], f16, tag=f"q{b}")
                nc.gpsimd.memset(t[:], 0.0)
                q.append(t)
                t = p_pool.tile([N_COMBOS, 128], f16, tag=f"pc{b}")
                nc.vector.tensor_copy(t[:], cSZB[:, b * 128 : (b + 1) * 128])
                pc.append(t)

            def tt(engine, out, a, bb, op):
                if engine == "pool":
                    nc.gpsimd.tensor_tensor(out, a, bb, op)
                else:
                    nc.vector.tensor_tensor(out, a, bb, op)

            # ---- software-pipelined iteration emission ----
            # b1 runs K stages behind b0 so its compute fills b0's latency gaps.
            NSTG = 14
            K_OFF = 7
            temps = [dict(), dict()]

            def emit(it, b, s):
                T = temps[b]
                if s == 0:
                    if it > 0:
                        T["w"] = sc_pool.tile([128, FW], f16, tag=f"w{b}", name=f"w{b}")
                        tt(CFG["w"], T["w"][:], q[b][:], naZ[b][:], ALU.add)
                elif s == 1:
                    if it > 0:
                        T["h"] = sc_pool.tile([128, FW], f16, tag=f"h{b}", name=f"h{b}")
                        tt(CFG["h"], T["h"][:], T["w"][:], aeb[b][:], ALU.add)
                elif s == 2:
                    if it > 0:
                        T["qn"] = q_pool.tile([128, FW], f16, tag=f"q{b}", name=f"qn{b}")
                        if CFG["qn"] == "act":
                            nc.scalar.activation(T["qn"][:], T["h"][:], AF.Relu)
                        else:
                            nc.vector.tensor_scalar_max(T["qn"][:], T["h"][:], 0.0)
                    else:
                        T["qn"] = q[b]
                elif s == 3:
                    if it > 0:
                        T["psT"] = ps_T.tile([128, FW], f16, tag=f"pT{b}", name=f"pT{b}")
                        for c in range(NF):
                            nc.tensor.transpose(
                                T["psT"][:, c * 128 : (c + 1) * 128],
                                aeb[b][:, c * 128 : (c + 1) * 128],
                                I16[:],
                            )
                elif s == 4:
                    T["aebT"] = sc_pool.tile([128, FW], f16, tag=f"aebT{b}", name=f"aebT{b}")
                    if it == 0:
                        # NAEB_0^T = alpha * Z^T, straight from the MLP's T-layout output
                        for c in range(NF):
                            if c % 2 == 0:
                                nc.scalar.activation(
                                    T["aebT"][:, c * 128 : (c + 1) * 128],
                                    zt[c][:, b * 128 : (b + 1) * 128].bitcast(f32),
                                    AF.Copy, scale=alpha,
                                )
                            else:
                                nc.vector.tensor_scalar(
                                    T["aebT"][:, c * 128 : (c + 1) * 128],
                                    zt[c][:, b * 128 : (b + 1) * 128].bitcast(f32),
                                    alpha, 0.0, op0=ALU.mult, op1=ALU.add,
                                )
                    elif CFG["tev"] == "act":
                        nc.scalar.activation(T["aebT"][:], T["psT"][:], AF.Copy)
                    elif CFG["tev"] == "dve":
                        nc.vector.tensor_copy(T["aebT"][:], T["psT"][:])
                    else:
                        nc.scalar.activation(T["aebT"][:, 0:256], T["psT"][:, 0:256], AF.Copy)
                        nc.vector.tensor_copy(T["aebT"][:, 256:512], T["psT"][:, 256:512])
                elif s == 5:
                    T["ps1"] = ps_y1.tile([N_COMBOS, 128], f32, tag="py", name=f"py{b}")
                    nc.tensor.matmul(T["ps1"][:], I64[:], pc[b][:], start=True, stop=False)
                    for c in range(NF):
                        nc.tensor.matmul(
                            T["ps1"][:],
                            ST16[:, c * 64 : (c + 1) * 64],
                            T["aebT"][:, c * 128 : (c + 1) * 128],
                            start=False, stop=(c == NF - 1),
                        )
                elif s == 6:
                    T["p"] = p_pool.tile([N_COMBOS, 128], f16, tag=f"p{b}", name=f"p{b}")
                    if CFG["p"] == "act":
                        nc.scalar.activation(T["p"][:], T["ps1"][:], AF.Relu)
                    else:
                        nc.vector.tensor_scalar_max(T["p"][:], T["ps1"][:], 0.0)
                elif s == 7:
                    if it == N_ITERS - 1:
                        return
                    T["pcn"] = p_pool.tile([N_COMBOS, 128], f16, tag=f"pc{b}", name=f"pc{b}")
                    if CFG["pc"] == "pool":
                        nc.gpsimd.tensor_tensor(
                            T["pcn"][:], T["p"][:], cSZB[:, b * 128 : (b + 1) * 128], ALU.add
                        )
                    else:
                        nc.vector.tensor_tensor(
                            T["pcn"][:], T["p"][:], cSZB[:, b * 128 : (b + 1) * 128], ALU.add
                        )
                elif s == 8:
                    T["ns3"] = ps_3.tile([128, FW], f32, tag=f"p3{b}", name=f"p3{b}")
                    nc.tensor.matmul(T["ns3"][:], nI16[:], E[b][:], start=True, stop=False)
                    if it > 0:
                        nc.tensor.matmul(T["ns3"][:], nAI16[:], T["qn"][:], start=False, stop=False)
                    nc.tensor.matmul(T["ns3"][:], T["p"][:], AS16[:], start=False, stop=True)
                elif s == 9:
                    T["n2"] = sc_pool.tile([128, 1], f32, tag=f"n2{b}", name=f"n2{b}")
                    dsq = sc_pool.tile([128, FW], f32, tag=f"dsq{b}", name=f"dsq{b}")
                    nc.scalar.activation(dsq[:], T["ns3"][:], AF.Square, accum_out=T["n2"][:])
                    T["nr"] = sc_pool.tile([128, 1], f32, tag=f"nr{b}", name=f"nr{b}")
                    nc.scalar.activation(
                        T["nr"][:], T["n2"][:], AF.Abs_reciprocal_sqrt, scale=dsq_scale, bias=1e-6
                    )
                    T["ns"] = sc_pool.tile([128, 1], f32, tag=f"ns{b}", name=f"ns{b}")
                    nc.vector.tensor_scalar(
                        T["ns"][:], T["nr"][:], 1.0, 0.0, op0=ALU.subtract, op1=ALU.min
                    )
                    T["s2p"] = sc_pool.tile([128, 1], f32, tag=f"s2p{b}", name=f"s2p{b}")
                    nc.vector.tensor_scalar(
                        T["s2p"][:], T["ns"][:], -2.0, 0.0, op0=ALU.mult, op1=ALU.add
                    )
                elif s == 10:
                    pass
                elif s == 11:
                    if it < N_ITERS - 1:
                        T["aebn"] = aeb_pool.tile([128, FW], f16, tag=f"aeb{b}", name=f"aebn{b}")
                        nc.vector.affine_then_add(
                            T["aebn"][:], T["ns3"][:], E[b][:], scale=T["s2p"][:], bias=-atau
                        )
                elif s == 12:
                    if it < N_ITERS - 1:
                        T["En"] = e_pool.tile([128, FW], f16, tag=f"e{b}", name=f"en{b}")
                        nc.scalar.activation(
                            T["En"][:], T["ns3"][:], AF.Copy, scale=T["ns"][:], bias=atau
                        )
                    else:
                        # x = Z + (ns/alpha)*NS3 directly from the final PSUM
                        nsa = sc_pool.tile([128, 1], f32, tag=f"nsa{b}", name=f"nsa{b}")
                        nc.vector.tensor_scalar(
                            nsa[:], T["ns"][:], 1.0 / alpha, 0.0, op0=ALU.mult, op1=ALU.add
                        )
                        xout = sc_pool.tile([128, FW], f32, tag=f"xo{b}")
                        nc.vector.affine_then_add(
                            xout[:], T["ns3"][:], Zf[b][:], scale=nsa[:], bias=0.0
                        )
                        nc.sync.dma_start(d_out.ap()[b * 128 : (b + 1) * 128, :], xout[:])
                elif s == 13:
                    if it < N_ITERS - 1:
                        E[b], aeb[b], q[b] = T["En"], T["aebn"], T["qn"]
                        if "pcn" in T:
                            pc[b] = T["pcn"]

            total = N_ITERS * NSTG
            for gs in range(total + K_OFF):
                if gs < total:
                    emit(gs // NSTG, 0, gs % NSTG)
                g1 = gs - K_OFF
                if 0 <= g1 < total:
                    emit(g1 // NSTG, 1, g1 % NSTG)


    nc.finalize()
    return nc


def _get_nc(S: np.ndarray):
    key = hash(S.tobytes())
    if key not in _BUILD_CACHE:
        L = _power_L(S)
        tau = 0.9 / L
        sigma = 0.9 / L
        _BUILD_CACHE[key] = (_build_nc(tau, sigma), tau, sigma)
    return _BUILD_CACHE[key]


def _make_in_maps(X, W1, b1, W2, b2, W3, b3, S, tau, sigma):
    f32 = np.float32
    alpha = np.float32(tau) * np.float32(sigma)
    a16 = np.float16(alpha).astype(f32)
    Xflat = np.ascontiguousarray(X.reshape(B_FULL, N_COMBOS)).astype(f32)
    S = S.astype(f32)
    # aST packed: alpha * S.T chunks [128, 64] side by side -> [128, 256]
    aST_full = (alpha * S.T).astype(f32)  # [512, 64]
    aST = np.ascontiguousarray(
        np.concatenate([aST_full[c * 128 : (c + 1) * 128, :] for c in range(NF)], axis=1)
    )
    ST16_full = (-S.T).astype(np.float16)  # negated: state is alpha*(Z-xbar)
    ST16 = np.ascontiguousarray(
        np.concatenate([ST16_full[c * 128 : (c + 1) * 128, :] for c in range(NF)], axis=1)
    )
    AS16 = np.ascontiguousarray((a16 * S).astype(np.float16))
    I128 = np.eye(128, dtype=f32)
    shared = {
        "w1": np.ascontiguousarray(W1.astype(f32)),
        "b1r": np.ascontiguousarray(b1.reshape(8, 128).T).astype(f32),
        "w2": np.ascontiguousarray(W2.astype(np.float16)),
        "b2r": np.ascontiguousarray(b2.reshape(8, 128).T).astype(f32),
        "w3": np.ascontiguousarray(W3.astype(np.float16)),
        "b3r": np.ascontiguousarray(b3.reshape(4, 128).T).astype(f32),
        "ast": aST,
        "st16": ST16,
        "as16": AS16,
        "nai16": np.ascontiguousarray((-a16 * I128).astype(np.float16)),
        "ni16": np.ascontiguousarray((-I128).astype(np.float16)),
        "i64_16": np.eye(N_COMBOS, dtype=np.float16),
        "i16": I128.astype(np.float16),
        "identr": I128,
    }
    in_maps = []
    for c in range(N_CORES):
        xt = np.ascontiguousarray(Xflat[c * BC : (c + 1) * BC, :].T)
        in_maps.append({**shared, "xt": xt})
    return in_maps


def kernel(X, W1, b1, W2, b2, W3, b3, S, batch_size):
    from concourse.bass_utils import run_bass_kernel_spmd

    X = np.asarray(X)
    S = np.asarray(S)
    nc, tau, sigma = _get_nc(np.ascontiguousarray(S.astype(np.float32)))
    in_maps = _make_in_maps(
        X,
        np.asarray(W1),
        np.asarray(b1),
        np.asarray(W2),
        np.asarray(b2),
        np.asarray(W3),
        np.asarray(b3),
        S,
        tau,
        sigma,
    )
    res = run_bass_kernel_spmd(nc, in_maps, core_ids=list(range(N_CORES)))
    out = np.concatenate([res.results[c]["out"] for c in range(N_CORES)], axis=0)
    return out.astype(np.float32)
